# revision 22
# baseline (speedup 1.0000x reference)
"""AggrGATConv Trainium2 hybrid: v2 pipeline + int16 message stream.

Design:
  inv-1 (device, node-sharded): h = feat @ W (split-bf16 exact), el/er tables.
  host (index-only): sort nodes by in-degree desc; octet k = 1024 consecutive
    sorted nodes -> 8 windows of 128 (one per core, snaked), padded to the
    octet max degree T_k. Row p of window k IS dst node -> the scatter matrix
    is the identity (no one-hot build, no dloc). Padding slots get el=-1e4
    so exp()=0. Host gathers h32[src]/el[src] rows (data movement only).
  inv-2 (device, edge-sharded, single pass): per window:
    lg = el + er_bcast; expE = max(exp(lg), exp(0.2 lg));
    s = reduce_t(expE); r = 1/max(s,eps);
    wmsg = hch * expE (fp32); U = sum_t wmsg_t via PE matmul with
    lhsT = 0.25*I (PSUM accumulate);
    out = sum_h(U_h * r_h) + bias_mean  (0.25 head-mean folded into I).
"""
import sys
import types
import contextlib
import ctypes
import os

import numpy as np

import concourse.bacc as bacc
import concourse.tile as tile
import concourse.mybir as mybir
from concourse.bass_utils import run_bass_kernel_spmd

# ---------------- constants (hardcoded per problem spec) ----------------
N = 100000
E = 1600000
IN = 128
H, D = 4, 32
HD = H * D  # 128
NEG = 0.2
NCORES = 8
P = 128
K_WIN = 98                       # octets: 100352 / 1024
N_PAD = NCORES * K_WIN * P       # 100352
NODES_PER_CORE = K_WIN * P       # 12544
PAD_LOGIT = -100.0  # exp(-100)=0, exp(-20)=2e-9; stays in ACT table range

f32 = mybir.dt.float32
i32 = mybir.dt.int32
i16 = mybir.dt.int16
QMAX = 32767.0
Exp = mybir.ActivationFunctionType.Exp
Copy = mybir.ActivationFunctionType.Copy
Add = mybir.AluOpType.add
Mult = mybir.AluOpType.mult
Max = mybir.AluOpType.max


def _install_ntff_shim():
    """antenv.axon_hooks is absent in this image; provide the ctypes hook so
    trace=True works (used by test harness; harmless otherwise)."""
    if "antenv.axon_hooks" in sys.modules:
        return
    try:
        lib = ctypes.CDLL("/opt/axon/libaxon_pjrt.so")
        if not hasattr(lib, "axon_start_nrt_profile"):
            raise OSError("no symbol")
        lib.axon_start_nrt_profile.argtypes = [
            ctypes.POINTER(ctypes.c_int64), ctypes.c_size_t]
        lib.axon_start_nrt_profile.restype = ctypes.c_int64
        lib.axon_stop_nrt_profile.argtypes = [ctypes.c_char_p]
        lib.axon_stop_nrt_profile.restype = ctypes.c_int64

        @contextlib.contextmanager
        def _hook(output_dir, device_ids):
            import jax
            jax.devices()
            if device_ids:
                ids = (ctypes.c_int64 * len(device_ids))(*device_ids)
                rc = lib.axon_start_nrt_profile(ids, len(device_ids))
            else:
                rc = lib.axon_start_nrt_profile(None, 0)
            if rc != 0:
                raise RuntimeError(f"axon_start_nrt_profile rc={rc}")
            try:
                yield
            finally:
                n = lib.axon_stop_nrt_profile(str(output_dir).encode())
                print(f"profile: {n} file(s) -> {output_dir}", file=sys.stderr)

        hook = _hook
    except OSError:
        hook = None
    mod = types.ModuleType("antenv.axon_hooks")
    mod.get_axon_ntff_profile_hook = lambda: hook
    mod.set_axon_ntff_profile_hook = lambda h: None
    sys.modules["antenv.axon_hooks"] = mod


_install_ntff_shim()


# ---------------- invocation 1: node tables + int16 quantization ---------
def _build_inv1():
    nc = bacc.Bacc("TRN2", target_bir_lowering=False, debug=False,
                   num_devices=NCORES)
    featT = nc.declare_dram_parameter("featT", [P, NODES_PER_CORE], f32,
                                      isOutput=False)
    W_in = nc.declare_dram_parameter("W", [IN, HD], f32, isOutput=False)
    WT_in = nc.declare_dram_parameter("WT", [HD, IN], f32, isOutput=False)
    Al_in = nc.declare_dram_parameter("Al", [HD, 4], f32, isOutput=False)
    Ar_in = nc.declare_dram_parameter("Ar", [HD, 4], f32, isOutput=False)
    q_out = nc.declare_dram_parameter("q_out", [P, K_WIN * HD], i16,
                                      isOutput=True)
    elr_out = nc.declare_dram_parameter("elr_out", [P, K_WIN * 8], f32,
                                        isOutput=True)
    sg_out = nc.declare_dram_parameter("sg_out", [P, K_WIN * 4], f32,
                                       isOutput=True)

    with tile.TileContext(nc) as tc:
        with tc.tile_pool(name="cst", bufs=1) as cst, \
             tc.tile_pool(name="sb", bufs=3) as sb, \
             tc.tile_pool(name="ps", bufs=3, space="PSUM") as ps, \
             tc.tile_pool(name="psw", bufs=1, space="PSUM") as psw:

            wt_sb = cst.tile([HD, IN], f32, tag="wt")
            nc.sync.dma_start(out=wt_sb[:], in_=WT_in[:])
            al_sb = cst.tile([HD, 4], f32, tag="al")
            nc.sync.dma_start(out=al_sb[:], in_=Al_in[:])
            ar_sb = cst.tile([HD, 4], f32, tag="ar")
            nc.sync.dma_start(out=ar_sb[:], in_=Ar_in[:])

            wlr = cst.tile([IN, 136], f32, tag="wlr")
            nc.sync.dma_start(out=wlr[:, 0:HD], in_=W_in[:])
            wl_ps = psw.tile([IN, 8], f32, tag="wlp")
            nc.tensor.matmul(out=wl_ps[:, 0:4], lhsT=wt_sb[:], rhs=al_sb[:],
                             start=True, stop=True)
            nc.tensor.matmul(out=wl_ps[:, 4:8], lhsT=wt_sb[:], rhs=ar_sb[:],
                             start=True, stop=True)
            nc.scalar.activation(out=wlr[:, 128:136], in_=wl_ps[:],
                                 func=Copy)

            CH = 14
            n_chunks = NODES_PER_CORE // (P * CH)
            for c in range(n_chunks):
                ft = sb.tile([P, CH * P], f32, tag="ft")
                nc.sync.dma_start(
                    out=ft[:], in_=featT[:, c * CH * P:(c + 1) * CH * P])
                hsb = sb.tile([P, CH * 136], f32, tag="hsb")
                for t in range(CH):
                    hp = ps.tile([P, 136], f32, tag="hp")
                    nc.tensor.matmul(out=hp[:],
                                     lhsT=ft[:, t * P:(t + 1) * P],
                                     rhs=wlr[:], start=True, stop=True)
                    if t % 2 == 0:
                        nc.scalar.activation(
                            out=hsb[:, t * 136:(t + 1) * 136], in_=hp[:],
                            func=Copy)
                    else:
                        nc.vector.tensor_copy(hsb[:, t * 136:(t + 1) * 136],
                                              hp[:])
                ga = hsb[:].rearrange("p (c f) -> p c f", c=CH)
                hview = ga[:, :, 0:128].rearrange(
                    "p c (hh d) -> p c hh d", hh=H)
                m4 = sb.tile([P, CH * 4], f32, tag="m4")
                nc.vector.tensor_reduce(
                    out=m4[:].rearrange("p (c h) -> p c h", c=CH),
                    in_=hview, axis=mybir.AxisListType.X, op=Max,
                    apply_absolute_value=True)
                # sg table carries the 1/H fold for the head-mean
                sgc = sb.tile([P, CH * 4], f32, tag="sgc")
                nc.vector.tensor_scalar_mul(sgc[:], m4[:], 1.0 / (QMAX * H))
                minv = sb.tile([P, CH * 4], f32, tag="minv")
                nc.vector.reciprocal(minv[:], m4[:])
                sinv = sb.tile([P, CH * 4], f32, tag="sinv")
                nc.vector.tensor_scalar_mul(sinv[:], minv[:], QMAX)
                q16 = sb.tile([P, CH * HD], i16, tag="q16")
                nc.vector.tensor_tensor(
                    out=q16[:].rearrange("p (c hh d) -> p c hh d",
                                         c=CH, hh=H),
                    in0=hview,
                    in1=sinv[:].rearrange("p (c h) -> p c h", c=CH)
                        .unsqueeze(3).to_broadcast([P, CH, H, D]),
                    op=Mult)
                elrs = sb.tile([P, CH * 8], f32, tag="elrs")
                nc.gpsimd.tensor_copy(
                    elrs[:].rearrange("p (c e) -> p c e", c=CH),
                    ga[:, :, 128:136])
                nc.gpsimd.dma_start(
                    out=q_out[:, c * CH * HD:(c + 1) * CH * HD], in_=q16[:])
                nc.gpsimd.dma_start(
                    out=elr_out[:, c * CH * 8:(c + 1) * CH * 8], in_=elrs[:])
                nc.gpsimd.dma_start(
                    out=sg_out[:, c * CH * 4:(c + 1) * CH * 4], in_=sgc[:])
    nc.compile()
    return nc


# ---------------- invocation 2: identity-layout edge aggregation ----------
def _build_inv2(Ts, mm_pairs=True, use_gpsimd=True):
    """Ts = per-window tile counts (desc), uniform across cores."""
    Ts = list(Ts)
    CAP = sum(Ts) * P           # hch cols
    CAP4 = sum(Ts) * 4          # el cols
    TMAX = max(max(Ts), 1)
    nc = bacc.Bacc("TRN2", target_bir_lowering=False, debug=False,
                   num_devices=NCORES)
    hsrc = nc.declare_dram_parameter("hsrc", [P, CAP], i16, isOutput=False)
    sg_d = nc.declare_dram_parameter("sgs", [P, CAP4], f32, isOutput=False)
    el_d = nc.declare_dram_parameter("el", [P, CAP4], f32, isOutput=False)
    er_d = nc.declare_dram_parameter("er", [P, K_WIN * 4], f32,
                                     isOutput=False)
    ident_d = nc.declare_dram_parameter("ident", [P, P], f32, isOutput=False)
    bias_in = nc.declare_dram_parameter("bias", [1, HD], f32, isOutput=False)
    out_d = nc.declare_dram_parameter("out", [P, K_WIN * D], f32,
                                      isOutput=True)

    Exp = mybir.ActivationFunctionType.Exp
    Copy = mybir.ActivationFunctionType.Copy
    Add = mybir.AluOpType.add
    Mult = mybir.AluOpType.mult
    Max = mybir.AluOpType.max

    with tile.TileContext(nc) as tc:
        with tc.tile_pool(name="cst", bufs=1) as cst, \
             tc.tile_pool(name="ld", bufs=3) as ld, \
             tc.tile_pool(name="wk", bufs=3) as wk, \
             tc.tile_pool(name="fl", bufs=3) as fl, \
             tc.tile_pool(name="ps", bufs=4, space="PSUM") as ps, \
             tc.tile_pool(name="psb", bufs=1, space="PSUM") as psb:

            # constants
            ident = cst.tile([P, P], f32, tag="ident")
            nc.sync.dma_start(out=ident[:], in_=ident_d[:])
            er_all = cst.tile([P, K_WIN * 4], f32, tag="erall")
            nc.sync.dma_start(out=er_all[:], in_=er_d[:])
            bias_sb = cst.tile([1, HD], f32, tag="brow")
            nc.sync.dma_start(out=bias_sb[:], in_=bias_in[:])
            bias_m = cst.tile([1, D], f32, tag="bm")
            nc.vector.tensor_reduce(
                out=bias_m[:],
                in_=bias_sb[0:1, :].rearrange("p (h d) -> p d h", h=H),
                axis=mybir.AxisListType.X, op=Add)
            nc.vector.tensor_scalar_mul(bias_m[:], bias_m[:], 1.0 / H)
            ones1 = cst.tile([1, P], f32, tag="ones")
            nc.vector.memset(ones1[:], 1.0)
            bias_ps = psb.tile([P, D], f32, tag="bps")
            nc.tensor.matmul(out=bias_ps[:], lhsT=ones1[:], rhs=bias_m[:],
                             start=True, stop=True)
            bias_bc = cst.tile([P, D], f32, tag="bbc")
            nc.vector.tensor_copy(bias_bc[:], bias_ps[:])

            offs = []
            _o = 0
            for _T in Ts:
                offs.append(_o)
                _o += _T

            # software-pipelined loop: window j's DMA + logit/exp chain
            # ("front") is emitted before window k<j's heavy body ("back"),
            # hiding the ACT exp round-trip under the previous multiply.
            pend = {}
            pend_sg = {}

            def front(j):
                T = Ts[j]
                off = offs[j]
                KW = T * P
                hch = ld.tile([P, TMAX * P], i16, tag="hch")
                KW2 = (T // 2) * P
                if KW2 > 0:
                    nc.sync.dma_start(
                        out=hch[:, 0:KW2],
                        in_=hsrc[:, off * P:off * P + KW2])
                    nc.sync.dma_start(
                        out=hch[:, KW2:KW],
                        in_=hsrc[:, off * P + KW2:off * P + KW])
                else:
                    nc.sync.dma_start(
                        out=hch[:, 0:KW],
                        in_=hsrc[:, off * P:off * P + KW])
                elch = ld.tile([P, TMAX * 4], f32, tag="elch")
                nc.gpsimd.dma_start(
                    out=elch[:, 0:T * 4],
                    in_=el_d[:, off * 4:off * 4 + T * 4])
                sgch = ld.tile([P, TMAX * 4], f32, tag="sgch")
                nc.gpsimd.dma_start(
                    out=sgch[:, 0:T * 4],
                    in_=sg_d[:, off * 4:off * 4 + T * 4])
                pend_sg[j] = sgch
                lg = fl.tile([P, TMAX * 4], f32, tag="lg")
                nc.vector.tensor_tensor(
                    out=lg[:, 0:T * 4].rearrange("p (t h) -> p t h", t=T),
                    in0=elch[:, 0:T * 4].rearrange("p (t h) -> p t h", t=T),
                    in1=er_all[:, j * 4:(j + 1) * 4].unsqueeze(1)
                        .to_broadcast([P, T, 4]),
                    op=Add)
                e1 = fl.tile([P, TMAX * 4], f32, tag="e1")
                nc.scalar.activation(out=e1[:, 0:T * 4], in_=lg[:, 0:T * 4],
                                     func=Exp)
                e2 = fl.tile([P, TMAX * 4], f32, tag="e2")
                nc.scalar.activation(out=e2[:, 0:T * 4], in_=lg[:, 0:T * 4],
                                     scale=NEG, func=Exp)
                pend[j] = (hch, e1, e2)

            _first = next((j for j, _T in enumerate(Ts) if _T > 0), None)
            if _first is not None:
                front(_first)
            for k, T in enumerate(Ts):
                if T == 0:
                    # isolated octet: out = bias_mean
                    nc.sync.dma_start(
                        out=out_d[:, k * D:(k + 1) * D], in_=bias_bc[:])
                    continue
                jn = next((j for j in range(k + 1, len(Ts)) if Ts[j] > 0),
                          None)
                if jn is not None:
                    front(jn)
                hch, e1, e2 = pend.pop(k)
                sgch = pend_sg.pop(k)
                eng_a = nc.gpsimd if use_gpsimd else nc.vector

                expE = fl.tile([P, TMAX * 4], f32, tag="expE")
                nc.vector.tensor_tensor(out=expE[:, 0:T * 4],
                                        in0=e1[:, 0:T * 4],
                                        in1=e2[:, 0:T * 4], op=Max)
                esw = fl.tile([P, TMAX * 4], f32, tag="esw")
                nc.gpsimd.tensor_tensor(out=esw[:, 0:T * 4],
                                        in0=expE[:, 0:T * 4],
                                        in1=sgch[:, 0:T * 4], op=Mult)

                # s, r per head
                s4 = fl.tile([P, 4], f32, tag="s4")
                nc.vector.tensor_reduce(
                    out=s4[:],
                    in_=expE[:, 0:T * 4].rearrange("p (t h) -> p h t", t=T),
                    axis=mybir.AxisListType.X, op=Add)
                # host plants el=ln(1e-6) in each row's first padding slot,
                # so s >= 1e-6 always and no clamp op is needed here
                r4 = fl.tile([P, 4], f32, tag="r4")
                nc.vector.reciprocal(r4[:], s4[:])

                # wmsg = hch * expE (broadcast over D), split in two halves
                # so PE matmuls can start after the first half completes
                wmsg = wk.tile([P, TMAX * P], f32, tag="wmsg")
                Ta = max(1, T // 2)
                m_eng = nc.gpsimd if (k % 4 == 1) else nc.vector
                for (ta, tb) in ((0, Ta), (Ta, T)):
                    tn = tb - ta
                    if tn <= 0:
                        continue
                    m_eng.tensor_tensor(
                        out=wmsg[:, ta * P:tb * P].rearrange(
                            "p (t h d) -> p t h d", t=tn, h=H),
                        in0=hch[:, ta * P:tb * P].rearrange(
                            "p (t h d) -> p t h d", t=tn, h=H),
                        in1=esw[:, ta * 4:tb * 4].rearrange(
                            "p (t h) -> p t h", t=tn).unsqueeze(3)
                            .to_broadcast([P, tn, H, D]),
                        op=Mult)

                # U = sum_t wmsg_t  (identity matmul, PSUM accumulate;
                # 0.25 head-mean folded into ident)
                if mm_pairs:
                    best, ndve = None, 0
                    for n in range(0, T):
                        dve_t = 2220 + 1350 + (230 if n else 0) + 230 * n
                        pe_t = 244 * (T - n)
                        m = max(dve_t, pe_t)
                        if best is None or m < best:
                            best, ndve = m, n
                else:
                    ndve = 0
                npe = T - ndve
                acc = ps.tile([P, HD], f32, tag="acc")
                for t in range(npe):
                    nc.tensor.matmul(
                        out=acc[:], lhsT=ident[:],
                        rhs=wmsg[:, t * P:(t + 1) * P],
                        start=(t == 0), stop=(t == npe - 1))
                if ndve > 0:
                    if ndve == 1:
                        dsrc = wmsg[:, npe * P:(npe + 1) * P]
                    else:
                        udve = fl.tile([P, HD], f32, tag="udve")
                        nc.vector.tensor_tensor(
                            out=udve[:], in0=wmsg[:, npe * P:(npe + 1) * P],
                            in1=wmsg[:, (npe + 1) * P:(npe + 2) * P], op=Add)
                        for t in range(npe + 2, T):
                            nc.vector.tensor_tensor(
                                out=udve[:], in0=udve[:],
                                in1=wmsg[:, t * P:(t + 1) * P], op=Add)
                        dsrc = udve[:]
                    u128 = fl.tile([P, HD], f32, tag="u128")
                    # u128 = (dsrc * 0.25) + acc (0.25 fold lives in ident)
                    nc.vector.scalar_tensor_tensor(
                        out=u128[:], in0=dsrc, scalar=1.0, in1=acc[:],
                        op0=Mult, op1=Add)
                    usrc = u128
                else:
                    usrc = acc

                # flush: out = sum_h (U_h * r_h) + bias_mean
                un = fl.tile([P, HD], f32, tag="un")
                for hh in range(H):
                    nc.scalar.activation(
                        out=un[:, hh * D:(hh + 1) * D],
                        in_=usrc[:, hh * D:(hh + 1) * D],
                        func=Copy, scale=r4[:, hh:hh + 1])
                red = fl.tile([P, D], f32, tag="red")
                nc.vector.tensor_reduce(
                    out=red[:],
                    in_=un[:].rearrange("p (h d) -> p d h", h=H),
                    axis=mybir.AxisListType.X, op=Add)
                outt = fl.tile([P, D], f32, tag="outt")
                eng_a.tensor_tensor(out=outt[:], in0=red[:],
                                    in1=bias_bc[:], op=Add)
                nc.gpsimd.dma_start(out=out_d[:, k * D:(k + 1) * D],
                                    in_=outt[:])
    nc.compile()
    return nc


_INV1 = None
_INV2 = {}
LAST_EXEC_NS = None
LAST_EXEC_NS1 = None
LAST_EXEC_NS2 = None
_TRACE = bool(os.environ.get("GAT_TRACE"))


def kernel(feat, W, attn_l, attn_r, bias, src, dst):
    global _INV1, LAST_EXEC_NS, LAST_EXEC_NS1, LAST_EXEC_NS2
    feat = np.asarray(feat, dtype=np.float32)
    W = np.asarray(W, dtype=np.float32)
    attn_l = np.asarray(attn_l, dtype=np.float32)
    attn_r = np.asarray(attn_r, dtype=np.float32)
    bias = np.asarray(bias, dtype=np.float32)
    src = np.asarray(src, dtype=np.int32)
    dst = np.asarray(dst, dtype=np.int32)

    # ---------------- host: layout-only prep ----------------
    featT = np.zeros((IN, N_PAD), dtype=np.float32)
    featT[:, :N] = np.ascontiguousarray(feat.T)
    WT = np.ascontiguousarray(W.T)
    Al = np.zeros((HD, H), dtype=np.float32)
    Ar = np.zeros((HD, H), dtype=np.float32)
    for h in range(H):
        Al[h * D:(h + 1) * D, h] = attn_l[h]
        Ar[h * D:(h + 1) * D, h] = attn_r[h]

    # ---------------- inv-1: node tables ----------------
    if _INV1 is None:
        _INV1 = _build_inv1()
    in1 = []
    for c in range(NCORES):
        sl = slice(c * NODES_PER_CORE, (c + 1) * NODES_PER_CORE)
        in1.append({"featT": np.ascontiguousarray(featT[:, sl]),
                    "W": W, "WT": WT, "Al": Al, "Ar": Ar})
    res1 = run_bass_kernel_spmd(_INV1, in1, core_ids=list(range(NCORES)),
                                trace=_TRACE)
    LAST_EXEC_NS1 = res1.exec_time_ns
    q_full = np.concatenate(
        [r["q_out"].reshape(P, K_WIN, HD).transpose(1, 0, 2)
         .reshape(NODES_PER_CORE, HD) for r in res1.results], axis=0)
    sg_full = np.concatenate(
        [r["sg_out"].reshape(P, K_WIN, 4).transpose(1, 0, 2)
         .reshape(NODES_PER_CORE, 4) for r in res1.results], axis=0)
    elr_full = np.concatenate(
        [r["elr_out"].reshape(P, K_WIN, 8).transpose(1, 0, 2)
         .reshape(NODES_PER_CORE, 8) for r in res1.results], axis=0)

    # ---------------- host: identity-layout slotting (index ops only) -----
    deg = np.bincount(dst, minlength=N_PAD).astype(np.int64)
    order = np.argsort(-deg, kind="stable")
    rank = np.empty(N_PAD, dtype=np.int64)
    rank[order] = np.arange(N_PAD)
    k_of = rank >> 10                  # octet index (node -> window)
    within = rank & 1023
    c_of = within >> 7
    c_of = np.where(k_of & 1 == 1, NCORES - 1 - c_of, c_of)  # snake
    p_of = within & 127

    Ts = deg[order[::1024]]            # max deg per octet (sorted desc)
    Ts = np.maximum(Ts, 0)
    # schedule: start with the 8 smallest windows (fast pipeline ramp),
    # then big -> small
    sched = np.concatenate([np.arange(K_WIN - 8, K_WIN),
                            np.arange(K_WIN - 8)])
    oct_pos = np.empty(K_WIN, dtype=np.int64)
    oct_pos[sched] = np.arange(K_WIN)
    Ts_s = Ts[sched]
    key = tuple(int(t) for t in Ts_s)
    tile_off = np.zeros(K_WIN + 1, dtype=np.int64)
    np.cumsum(Ts_s, out=tile_off[1:])
    CAP = int(tile_off[-1]) * P
    w_of = oct_pos[k_of]               # node -> window position in schedule

    # per-edge slot: t = running count within dst node
    perm = np.argsort(dst, kind="stable")
    dstp = dst[perm]
    srcp = src[perm]
    estart = np.zeros(N_PAD + 1, dtype=np.int64)
    np.cumsum(np.bincount(dstp, minlength=N_PAD), out=estart[1:])
    t_of = np.arange(E, dtype=np.int64) - estart[dstp]
    ce = c_of[dstp]
    pe = p_of[dstp]
    tile_e = tile_off[w_of[dstp]] + t_of

    # gathers (pure data movement)
    n_tiles = int(tile_off[-1])
    hsrc_lay = np.zeros((NCORES, P, n_tiles, HD), dtype=np.int16)
    hsrc_lay[ce, pe, tile_e, :] = q_full[srcp]
    sg_lay = np.zeros((NCORES, P, n_tiles, 4), dtype=np.float32)
    sg_lay[ce, pe, tile_e, :] = sg_full[srcp]
    el_lay = np.full((NCORES, P, n_tiles, 4), PAD_LOGIT, dtype=np.float32)
    el_lay[ce, pe, tile_e, :] = elr_full[srcp][:, 0:4]
    # plant el=ln(1e-6) in each row's first padding slot so s >= 1e-6
    # (replaces the device-side max(s, eps) clamp; harmless: adds 1e-6
    # to s where real s >= ~0.2, and gives empty rows s=1e-6 with U=0)
    # value -69.08 = 5*ln(1e-6): through max(exp(x), exp(0.2x)) this
    # contributes exactly ~1e-6
    has_pad = deg < Ts_s[w_of]
    el_lay[c_of[has_pad], p_of[has_pad],
           (tile_off[w_of] + deg)[has_pad], :] = -69.07755

    er_lay = np.zeros((NCORES, P, K_WIN, 4), dtype=np.float32)
    nodes = np.arange(N_PAD)
    er_lay[c_of, p_of, w_of, :] = elr_full[nodes][:, 4:8]
    ident = np.eye(P).astype(np.float32)  # 1/H lives in the sg table

    # ---------------- inv-2: edge aggregation ----------------
    if key not in _INV2:
        _INV2[key] = _build_inv2(key, mm_pairs=True, use_gpsimd=False)
    in2 = []
    for c in range(NCORES):
        in2.append({"hsrc": hsrc_lay[c].reshape(P, CAP),
                    "sgs": sg_lay[c].reshape(P, n_tiles * 4),
                    "el": el_lay[c].reshape(P, n_tiles * 4),
                    "er": er_lay[c].reshape(P, K_WIN * 4),
                    "ident": ident,
                    "bias": bias.reshape(1, HD)})
    res2 = run_bass_kernel_spmd(_INV2[key], in2, core_ids=list(range(NCORES)),
                                trace=_TRACE)
    LAST_EXEC_NS2 = res2.exec_time_ns
    if LAST_EXEC_NS1 is not None and LAST_EXEC_NS2 is not None:
        LAST_EXEC_NS = LAST_EXEC_NS1 + LAST_EXEC_NS2
    out_full = np.zeros((N_PAD, D), dtype=np.float32)
    res_arr = np.stack([r["out"].reshape(P, K_WIN, D)
                        for r in res2.results])  # [c, p, k, d]
    out_full[nodes] = res_arr[c_of, p_of, w_of, :]
    return np.ascontiguousarray(out_full[:N])



# revision 23
# speedup vs baseline: 1.1744x; 1.1744x over previous
"""AggrGATConv Trainium2 kernel v2: identity-layout edge aggregation.

Design:
  inv-1 (device, node-sharded): h = feat @ W (split-bf16 exact), el/er tables.
  host (index-only): sort nodes by in-degree desc; octet k = 1024 consecutive
    sorted nodes -> 8 windows of 128 (one per core, snaked), padded to the
    octet max degree T_k. Row p of window k IS dst node -> the scatter matrix
    is the identity (no one-hot build, no dloc). Padding slots get el=-1e4
    so exp()=0. Host gathers h32[src]/el[src] rows (data movement only).
  inv-2 (device, edge-sharded, single pass): per window:
    lg = el + er_bcast; expE = max(exp(lg), exp(0.2 lg));
    s = reduce_t(expE); r = 1/max(s,eps);
    wmsg = hch * expE (fp32); U = sum_t wmsg_t via PE matmul with
    lhsT = 0.25*I (PSUM accumulate);
    out = sum_h(U_h * r_h) + bias_mean  (0.25 head-mean folded into I).
"""
import sys
import types
import contextlib
import ctypes
import os

import numpy as np

import concourse.bacc as bacc
import concourse.tile as tile
import concourse.mybir as mybir
from concourse.bass_utils import run_bass_kernel_spmd

# ---------------- constants (hardcoded per problem spec) ----------------
N = 100000
E = 1600000
IN = 128
H, D = 4, 32
HD = H * D  # 128
NEG = 0.2
NCORES = 8
P = 128
K_WIN = 98                       # octets: 100352 / 1024
N_PAD = NCORES * K_WIN * P       # 100352
NODES_PER_CORE = K_WIN * P       # 12544
PAD_LOGIT = -100.0  # exp(-100)=0, exp(-20)=2e-9; stays in ACT table range

f32 = mybir.dt.float32
i32 = mybir.dt.int32


def _install_ntff_shim():
    """antenv.axon_hooks is absent in this image; provide the ctypes hook so
    trace=True works (used by test harness; harmless otherwise)."""
    if "antenv.axon_hooks" in sys.modules:
        return
    try:
        lib = ctypes.CDLL("/opt/axon/libaxon_pjrt.so")
        if not hasattr(lib, "axon_start_nrt_profile"):
            raise OSError("no symbol")
        lib.axon_start_nrt_profile.argtypes = [
            ctypes.POINTER(ctypes.c_int64), ctypes.c_size_t]
        lib.axon_start_nrt_profile.restype = ctypes.c_int64
        lib.axon_stop_nrt_profile.argtypes = [ctypes.c_char_p]
        lib.axon_stop_nrt_profile.restype = ctypes.c_int64

        @contextlib.contextmanager
        def _hook(output_dir, device_ids):
            import jax
            jax.devices()
            if device_ids:
                ids = (ctypes.c_int64 * len(device_ids))(*device_ids)
                rc = lib.axon_start_nrt_profile(ids, len(device_ids))
            else:
                rc = lib.axon_start_nrt_profile(None, 0)
            if rc != 0:
                raise RuntimeError(f"axon_start_nrt_profile rc={rc}")
            try:
                yield
            finally:
                n = lib.axon_stop_nrt_profile(str(output_dir).encode())
                print(f"profile: {n} file(s) -> {output_dir}", file=sys.stderr)

        hook = _hook
    except OSError:
        hook = None
    mod = types.ModuleType("antenv.axon_hooks")
    mod.get_axon_ntff_profile_hook = lambda: hook
    mod.set_axon_ntff_profile_hook = lambda h: None
    sys.modules["antenv.axon_hooks"] = mod


_install_ntff_shim()


# ---------------- invocation 1: node tables ----------------
def _build_inv1():
    nc = bacc.Bacc("TRN2", target_bir_lowering=False, debug=False,
                   num_devices=NCORES)
    featT = nc.declare_dram_parameter("featT", [P, NODES_PER_CORE], f32,
                                      isOutput=False)
    W_in = nc.declare_dram_parameter("W", [IN, HD], f32, isOutput=False)
    WT_in = nc.declare_dram_parameter("WT", [HD, IN], f32, isOutput=False)
    Al_in = nc.declare_dram_parameter("Al", [HD, 4], f32, isOutput=False)
    Ar_in = nc.declare_dram_parameter("Ar", [HD, 4], f32, isOutput=False)
    # partition-major outputs: row p holds all its tiles contiguously
    # (fat DMA descriptors); host reshapes back to node-major for free.
    h_out = nc.declare_dram_parameter("h_out", [P, K_WIN * HD], f32,
                                      isOutput=True)
    elr_out = nc.declare_dram_parameter("elr_out", [P, K_WIN * 8], f32,
                                        isOutput=True)

    with tile.TileContext(nc) as tc:
        with tc.tile_pool(name="cst", bufs=1) as cst, \
             tc.tile_pool(name="sb", bufs=3) as sb, \
             tc.tile_pool(name="ps", bufs=3, space="PSUM") as ps, \
             tc.tile_pool(name="psw", bufs=1, space="PSUM") as psw:

            # WLR = [W | Wl | Wr] where Wl = W @ Al, Wr = W @ Ar
            wt_sb = cst.tile([HD, IN], f32, tag="wt")
            nc.sync.dma_start(out=wt_sb[:], in_=WT_in[:])
            al_sb = cst.tile([HD, 4], f32, tag="al")
            nc.sync.dma_start(out=al_sb[:], in_=Al_in[:])
            ar_sb = cst.tile([HD, 4], f32, tag="ar")
            nc.sync.dma_start(out=ar_sb[:], in_=Ar_in[:])

            wlr = cst.tile([IN, 136], f32, tag="wlr")
            nc.sync.dma_start(out=wlr[:, 0:HD], in_=W_in[:])
            wl_ps = psw.tile([IN, 8], f32, tag="wlp")
            nc.tensor.matmul(out=wl_ps[:, 0:4], lhsT=wt_sb[:], rhs=al_sb[:],
                             start=True, stop=True)
            nc.tensor.matmul(out=wl_ps[:, 4:8], lhsT=wt_sb[:], rhs=ar_sb[:],
                             start=True, stop=True)
            nc.scalar.activation(out=wlr[:, 128:136], in_=wl_ps[:],
                                 func=mybir.ActivationFunctionType.Copy)

            CH = 14  # tiles per chunk; 98 = 7 chunks of 14
            n_chunks = NODES_PER_CORE // (P * CH)
            for c in range(n_chunks):
                ft = sb.tile([P, CH * P], f32, tag="ft")
                nc.sync.dma_start(
                    out=ft[:], in_=featT[:, c * CH * P:(c + 1) * CH * P])
                hsb = sb.tile([P, CH * HD], f32, tag="hsb")
                esb = sb.tile([P, CH * 8], f32, tag="esb")
                for t in range(CH):
                    hp = ps.tile([P, 136], f32, tag="hp")
                    # fp32 matmul mode is exact (2 half-speed passes)
                    nc.tensor.matmul(out=hp[:],
                                     lhsT=ft[:, t * P:(t + 1) * P],
                                     rhs=wlr[:], start=True, stop=True)
                    if t % 2 == 0:
                        nc.scalar.activation(
                            out=hsb[:, t * HD:(t + 1) * HD], in_=hp[:, 0:HD],
                            func=mybir.ActivationFunctionType.Copy)
                    else:
                        nc.vector.tensor_copy(hsb[:, t * HD:(t + 1) * HD],
                                              hp[:, 0:HD])
                    nc.vector.tensor_copy(esb[:, t * 8:(t + 1) * 8],
                                          hp[:, 128:136])
                nc.gpsimd.dma_start(
                    out=h_out[:, c * CH * HD:(c + 1) * CH * HD],
                    in_=hsb[:])
                nc.gpsimd.dma_start(
                    out=elr_out[:, c * CH * 8:(c + 1) * CH * 8],
                    in_=esb[:])
    nc.compile()
    return nc


# ---------------- invocation 2: identity-layout edge aggregation ----------
def _build_inv2(Ts, mm_pairs=True, use_gpsimd=True):
    """Ts = per-window tile counts (desc), uniform across cores."""
    Ts = list(Ts)
    CAP = sum(Ts) * P           # hch cols
    CAP4 = sum(Ts) * 4          # el cols
    TMAX = max(max(Ts), 1)
    nc = bacc.Bacc("TRN2", target_bir_lowering=False, debug=False,
                   num_devices=NCORES)
    hsrc = nc.declare_dram_parameter("hsrc", [P, CAP], f32, isOutput=False)
    el_d = nc.declare_dram_parameter("el", [P, CAP4], f32, isOutput=False)
    er_d = nc.declare_dram_parameter("er", [P, K_WIN * 4], f32,
                                     isOutput=False)
    ident_d = nc.declare_dram_parameter("ident", [P, P], f32, isOutput=False)
    bias_in = nc.declare_dram_parameter("bias", [1, HD], f32, isOutput=False)
    out_d = nc.declare_dram_parameter("out", [P, K_WIN * D], f32,
                                      isOutput=True)

    Exp = mybir.ActivationFunctionType.Exp
    Copy = mybir.ActivationFunctionType.Copy
    Add = mybir.AluOpType.add
    Mult = mybir.AluOpType.mult
    Max = mybir.AluOpType.max

    with tile.TileContext(nc) as tc:
        with tc.tile_pool(name="cst", bufs=1) as cst, \
             tc.tile_pool(name="ld", bufs=3) as ld, \
             tc.tile_pool(name="wk", bufs=3) as wk, \
             tc.tile_pool(name="fl", bufs=3) as fl, \
             tc.tile_pool(name="ps", bufs=4, space="PSUM") as ps, \
             tc.tile_pool(name="psb", bufs=1, space="PSUM") as psb:

            # constants
            ident = cst.tile([P, P], f32, tag="ident")
            nc.sync.dma_start(out=ident[:], in_=ident_d[:])
            er_all = cst.tile([P, K_WIN * 4], f32, tag="erall")
            nc.sync.dma_start(out=er_all[:], in_=er_d[:])
            bias_sb = cst.tile([1, HD], f32, tag="brow")
            nc.sync.dma_start(out=bias_sb[:], in_=bias_in[:])
            bias_m = cst.tile([1, D], f32, tag="bm")
            nc.vector.tensor_reduce(
                out=bias_m[:],
                in_=bias_sb[0:1, :].rearrange("p (h d) -> p d h", h=H),
                axis=mybir.AxisListType.X, op=Add)
            nc.vector.tensor_scalar_mul(bias_m[:], bias_m[:], 1.0 / H)
            ones1 = cst.tile([1, P], f32, tag="ones")
            nc.vector.memset(ones1[:], 1.0)
            bias_ps = psb.tile([P, D], f32, tag="bps")
            nc.tensor.matmul(out=bias_ps[:], lhsT=ones1[:], rhs=bias_m[:],
                             start=True, stop=True)
            bias_bc = cst.tile([P, D], f32, tag="bbc")
            nc.vector.tensor_copy(bias_bc[:], bias_ps[:])

            offs = []
            _o = 0
            for _T in Ts:
                offs.append(_o)
                _o += _T

            # software-pipelined loop: window j's DMA + logit/exp chain
            # ("front") is emitted before window k<j's heavy body ("back"),
            # hiding the ACT exp round-trip under the previous multiply.
            pend = {}

            def front(j):
                T = Ts[j]
                off = offs[j]
                KW = T * P
                hch = ld.tile([P, TMAX * P], f32, tag="hch")
                KW2 = (T // 2) * P
                if KW2 > 0:
                    nc.sync.dma_start(
                        out=hch[:, 0:KW2],
                        in_=hsrc[:, off * P:off * P + KW2])
                    nc.sync.dma_start(
                        out=hch[:, KW2:KW],
                        in_=hsrc[:, off * P + KW2:off * P + KW])
                else:
                    nc.sync.dma_start(
                        out=hch[:, 0:KW],
                        in_=hsrc[:, off * P:off * P + KW])
                elch = ld.tile([P, TMAX * 4], f32, tag="elch")
                nc.gpsimd.dma_start(
                    out=elch[:, 0:T * 4],
                    in_=el_d[:, off * 4:off * 4 + T * 4])
                lg = fl.tile([P, TMAX * 4], f32, tag="lg")
                nc.vector.tensor_tensor(
                    out=lg[:, 0:T * 4].rearrange("p (t h) -> p t h", t=T),
                    in0=elch[:, 0:T * 4].rearrange("p (t h) -> p t h", t=T),
                    in1=er_all[:, j * 4:(j + 1) * 4].unsqueeze(1)
                        .to_broadcast([P, T, 4]),
                    op=Add)
                e1 = fl.tile([P, TMAX * 4], f32, tag="e1")
                nc.scalar.activation(out=e1[:, 0:T * 4], in_=lg[:, 0:T * 4],
                                     func=Exp)
                e2 = fl.tile([P, TMAX * 4], f32, tag="e2")
                nc.scalar.activation(out=e2[:, 0:T * 4], in_=lg[:, 0:T * 4],
                                     scale=NEG, func=Exp)
                pend[j] = (hch, e1, e2)

            _first = next((j for j, _T in enumerate(Ts) if _T > 0), None)
            if _first is not None:
                front(_first)
            for k, T in enumerate(Ts):
                if T == 0:
                    # isolated octet: out = bias_mean
                    nc.sync.dma_start(
                        out=out_d[:, k * D:(k + 1) * D], in_=bias_bc[:])
                    continue
                jn = next((j for j in range(k + 1, len(Ts)) if Ts[j] > 0),
                          None)
                if jn is not None:
                    front(jn)
                hch, e1, e2 = pend.pop(k)
                eng_a = nc.gpsimd if use_gpsimd else nc.vector

                expE = fl.tile([P, TMAX * 4], f32, tag="expE")
                eng_a.tensor_tensor(out=expE[:, 0:T * 4],
                                    in0=e1[:, 0:T * 4],
                                    in1=e2[:, 0:T * 4], op=Max)

                # s, r per head
                s4 = fl.tile([P, 4], f32, tag="s4")
                nc.vector.tensor_reduce(
                    out=s4[:],
                    in_=expE[:, 0:T * 4].rearrange("p (t h) -> p h t", t=T),
                    axis=mybir.AxisListType.X, op=Add)
                # host plants el=ln(1e-6) in each row's first padding slot,
                # so s >= 1e-6 always and no clamp op is needed here
                r4 = fl.tile([P, 4], f32, tag="r4")
                nc.vector.reciprocal(r4[:], s4[:])

                # wmsg = hch * expE (broadcast over D), split in two halves
                # so PE matmuls can start after the first half completes
                wmsg = wk.tile([P, TMAX * P], f32, tag="wmsg")
                Ta = max(1, T // 2)
                for (ta, tb) in ((0, Ta), (Ta, T)):
                    tn = tb - ta
                    if tn <= 0:
                        continue
                    nc.vector.tensor_tensor(
                        out=wmsg[:, ta * P:tb * P].rearrange(
                            "p (t h d) -> p t h d", t=tn, h=H),
                        in0=hch[:, ta * P:tb * P].rearrange(
                            "p (t h d) -> p t h d", t=tn, h=H),
                        in1=expE[:, ta * 4:tb * 4].rearrange(
                            "p (t h) -> p t h", t=tn).unsqueeze(3)
                            .to_broadcast([P, tn, H, D]),
                        op=Mult)

                # U = sum_t wmsg_t  (identity matmul, PSUM accumulate;
                # 0.25 head-mean folded into ident)
                if mm_pairs:
                    best, ndve = None, 0
                    for n in range(0, T):
                        dve_t = 2220 + 1350 + (230 if n else 0) + 230 * n
                        pe_t = 244 * (T - n)
                        m = max(dve_t, pe_t)
                        if best is None or m < best:
                            best, ndve = m, n
                else:
                    ndve = 0
                npe = T - ndve
                acc = ps.tile([P, HD], f32, tag="acc")
                for t in range(npe):
                    nc.tensor.matmul(
                        out=acc[:], lhsT=ident[:],
                        rhs=wmsg[:, t * P:(t + 1) * P],
                        start=(t == 0), stop=(t == npe - 1))
                if ndve > 0:
                    if ndve == 1:
                        dsrc = wmsg[:, npe * P:(npe + 1) * P]
                    else:
                        udve = fl.tile([P, HD], f32, tag="udve")
                        nc.vector.tensor_tensor(
                            out=udve[:], in0=wmsg[:, npe * P:(npe + 1) * P],
                            in1=wmsg[:, (npe + 1) * P:(npe + 2) * P], op=Add)
                        for t in range(npe + 2, T):
                            nc.vector.tensor_tensor(
                                out=udve[:], in0=udve[:],
                                in1=wmsg[:, t * P:(t + 1) * P], op=Add)
                        dsrc = udve[:]
                    u128 = fl.tile([P, HD], f32, tag="u128")
                    # u128 = (dsrc * 0.25) + acc (0.25 fold lives in ident)
                    nc.vector.scalar_tensor_tensor(
                        out=u128[:], in0=dsrc, scalar=0.25, in1=acc[:],
                        op0=Mult, op1=Add)
                    usrc = u128
                else:
                    usrc = acc

                # flush: out = sum_h (U_h * r_h) + bias_mean
                un = fl.tile([P, HD], f32, tag="un")
                for hh in range(H):
                    nc.scalar.activation(
                        out=un[:, hh * D:(hh + 1) * D],
                        in_=usrc[:, hh * D:(hh + 1) * D],
                        func=Copy, scale=r4[:, hh:hh + 1])
                red = fl.tile([P, D], f32, tag="red")
                nc.vector.tensor_reduce(
                    out=red[:],
                    in_=un[:].rearrange("p (h d) -> p d h", h=H),
                    axis=mybir.AxisListType.X, op=Add)
                outt = fl.tile([P, D], f32, tag="outt")
                eng_a.tensor_tensor(out=outt[:], in0=red[:],
                                    in1=bias_bc[:], op=Add)
                nc.gpsimd.dma_start(out=out_d[:, k * D:(k + 1) * D],
                                    in_=outt[:])
    nc.compile()
    return nc


_INV1 = None
_INV2 = {}
LAST_EXEC_NS = None
LAST_EXEC_NS1 = None
LAST_EXEC_NS2 = None
_TRACE = bool(os.environ.get("GAT_TRACE"))


def kernel(feat, W, attn_l, attn_r, bias, src, dst):
    global _INV1, LAST_EXEC_NS, LAST_EXEC_NS1, LAST_EXEC_NS2
    feat = np.asarray(feat, dtype=np.float32)
    W = np.asarray(W, dtype=np.float32)
    attn_l = np.asarray(attn_l, dtype=np.float32)
    attn_r = np.asarray(attn_r, dtype=np.float32)
    bias = np.asarray(bias, dtype=np.float32)
    src = np.asarray(src, dtype=np.int32)
    dst = np.asarray(dst, dtype=np.int32)

    # ---------------- host: layout-only prep ----------------
    featT = np.zeros((IN, N_PAD), dtype=np.float32)
    featT[:, :N] = np.ascontiguousarray(feat.T)
    WT = np.ascontiguousarray(W.T)
    Al = np.zeros((HD, H), dtype=np.float32)
    Ar = np.zeros((HD, H), dtype=np.float32)
    for h in range(H):
        Al[h * D:(h + 1) * D, h] = attn_l[h]
        Ar[h * D:(h + 1) * D, h] = attn_r[h]

    # ---------------- inv-1: node tables ----------------
    if _INV1 is None:
        _INV1 = _build_inv1()
    in1 = []
    for c in range(NCORES):
        sl = slice(c * NODES_PER_CORE, (c + 1) * NODES_PER_CORE)
        in1.append({"featT": np.ascontiguousarray(featT[:, sl]),
                    "W": W, "WT": WT, "Al": Al, "Ar": Ar})
    res1 = run_bass_kernel_spmd(_INV1, in1, core_ids=list(range(NCORES)),
                                trace=_TRACE)
    LAST_EXEC_NS1 = res1.exec_time_ns
    h_full = np.concatenate(
        [r["h_out"].reshape(P, K_WIN, HD).transpose(1, 0, 2)
         .reshape(NODES_PER_CORE, HD) for r in res1.results], axis=0)
    elr_full = np.concatenate(
        [r["elr_out"].reshape(P, K_WIN, 8).transpose(1, 0, 2)
         .reshape(NODES_PER_CORE, 8) for r in res1.results], axis=0)

    # ---------------- host: identity-layout slotting (index ops only) -----
    deg = np.bincount(dst, minlength=N_PAD).astype(np.int64)
    order = np.argsort(-deg, kind="stable")
    rank = np.empty(N_PAD, dtype=np.int64)
    rank[order] = np.arange(N_PAD)
    k_of = rank >> 10                  # octet index (node -> window)
    within = rank & 1023
    c_of = within >> 7
    c_of = np.where(k_of & 1 == 1, NCORES - 1 - c_of, c_of)  # snake
    p_of = within & 127

    Ts = deg[order[::1024]]            # max deg per octet (sorted desc)
    Ts = np.maximum(Ts, 0)
    # schedule: start with the 8 smallest windows (fast pipeline ramp),
    # then big -> small
    sched = np.concatenate([np.arange(K_WIN - 8, K_WIN),
                            np.arange(K_WIN - 8)])
    oct_pos = np.empty(K_WIN, dtype=np.int64)
    oct_pos[sched] = np.arange(K_WIN)
    Ts_s = Ts[sched]
    key = tuple(int(t) for t in Ts_s)
    tile_off = np.zeros(K_WIN + 1, dtype=np.int64)
    np.cumsum(Ts_s, out=tile_off[1:])
    CAP = int(tile_off[-1]) * P
    w_of = oct_pos[k_of]               # node -> window position in schedule

    # per-edge slot: t = running count within dst node
    perm = np.argsort(dst, kind="stable")
    dstp = dst[perm]
    srcp = src[perm]
    estart = np.zeros(N_PAD + 1, dtype=np.int64)
    np.cumsum(np.bincount(dstp, minlength=N_PAD), out=estart[1:])
    t_of = np.arange(E, dtype=np.int64) - estart[dstp]
    ce = c_of[dstp]
    pe = p_of[dstp]
    tile_e = tile_off[w_of[dstp]] + t_of

    # gathers (pure data movement)
    n_tiles = int(tile_off[-1])
    hsrc_lay = np.zeros((NCORES, P, n_tiles, HD), dtype=np.float32)
    hsrc_lay[ce, pe, tile_e, :] = h_full[srcp]
    el_lay = np.full((NCORES, P, n_tiles, 4), PAD_LOGIT, dtype=np.float32)
    el_lay[ce, pe, tile_e, :] = elr_full[srcp][:, 0:4]
    # plant el=ln(1e-6) in each row's first padding slot so s >= 1e-6
    # (replaces the device-side max(s, eps) clamp; harmless: adds 1e-6
    # to s where real s >= ~0.2, and gives empty rows s=1e-6 with U=0)
    # value -69.08 = 5*ln(1e-6): through max(exp(x), exp(0.2x)) this
    # contributes exactly ~1e-6
    has_pad = deg < Ts_s[w_of]
    el_lay[c_of[has_pad], p_of[has_pad],
           (tile_off[w_of] + deg)[has_pad], :] = -69.07755

    er_lay = np.zeros((NCORES, P, K_WIN, 4), dtype=np.float32)
    nodes = np.arange(N_PAD)
    er_lay[c_of, p_of, w_of, :] = elr_full[nodes][:, 4:8]
    ident = (0.25 * np.eye(P)).astype(np.float32)

    # ---------------- inv-2: edge aggregation ----------------
    if key not in _INV2:
        _INV2[key] = _build_inv2(key, mm_pairs=True, use_gpsimd=False)
    in2 = []
    for c in range(NCORES):
        in2.append({"hsrc": hsrc_lay[c].reshape(P, CAP),
                    "el": el_lay[c].reshape(P, n_tiles * 4),
                    "er": er_lay[c].reshape(P, K_WIN * 4),
                    "ident": ident,
                    "bias": bias.reshape(1, HD)})
    res2 = run_bass_kernel_spmd(_INV2[key], in2, core_ids=list(range(NCORES)),
                                trace=_TRACE)
    LAST_EXEC_NS2 = res2.exec_time_ns
    if LAST_EXEC_NS1 is not None and LAST_EXEC_NS2 is not None:
        LAST_EXEC_NS = LAST_EXEC_NS1 + LAST_EXEC_NS2
    out_full = np.zeros((N_PAD, D), dtype=np.float32)
    res_arr = np.stack([r["out"].reshape(P, K_WIN, D)
                        for r in res2.results])  # [c, p, k, d]
    out_full[nodes] = res_arr[c_of, p_of, w_of, :]
    return np.ascontiguousarray(out_full[:N])



# revision 24
# speedup vs baseline: 1.2027x; 1.0240x over previous
"""AggrGATConv Trainium2 kernel v2: identity-layout edge aggregation.

Design:
  inv-1 (device, node-sharded): h = feat @ W (split-bf16 exact), el/er tables.
  host (index-only): sort nodes by in-degree desc; octet k = 1024 consecutive
    sorted nodes -> 8 windows of 128 (one per core, snaked), padded to the
    octet max degree T_k. Row p of window k IS dst node -> the scatter matrix
    is the identity (no one-hot build, no dloc). Padding slots get el=-1e4
    so exp()=0. Host gathers h32[src]/el[src] rows (data movement only).
  inv-2 (device, edge-sharded, single pass): per window:
    lg = el + er_bcast; expE = max(exp(lg), exp(0.2 lg));
    s = reduce_t(expE); r = 1/max(s,eps);
    wmsg = hch * expE (fp32); U = sum_t wmsg_t via PE matmul with
    lhsT = 0.25*I (PSUM accumulate);
    out = sum_h(U_h * r_h) + bias_mean  (0.25 head-mean folded into I).
"""
import sys
import types
import contextlib
import ctypes
import os

import numpy as np

import concourse.bacc as bacc
import concourse.tile as tile
import concourse.mybir as mybir
from concourse.bass_utils import run_bass_kernel_spmd

# ---------------- constants (hardcoded per problem spec) ----------------
N = 100000
E = 1600000
IN = 128
H, D = 4, 32
HD = H * D  # 128
NEG = 0.2
NCORES = 8
P = 128
K_WIN = 98                       # octets: 100352 / 1024
N_PAD = NCORES * K_WIN * P       # 100352
NODES_PER_CORE = K_WIN * P       # 12544
PAD_LOGIT = -100.0  # exp(-100)=0, exp(-20)=2e-9; stays in ACT table range

f32 = mybir.dt.float32
i32 = mybir.dt.int32


def _install_ntff_shim():
    """antenv.axon_hooks is absent in this image; provide the ctypes hook so
    trace=True works (used by test harness; harmless otherwise)."""
    if "antenv.axon_hooks" in sys.modules:
        return
    try:
        lib = ctypes.CDLL("/opt/axon/libaxon_pjrt.so")
        if not hasattr(lib, "axon_start_nrt_profile"):
            raise OSError("no symbol")
        lib.axon_start_nrt_profile.argtypes = [
            ctypes.POINTER(ctypes.c_int64), ctypes.c_size_t]
        lib.axon_start_nrt_profile.restype = ctypes.c_int64
        lib.axon_stop_nrt_profile.argtypes = [ctypes.c_char_p]
        lib.axon_stop_nrt_profile.restype = ctypes.c_int64

        @contextlib.contextmanager
        def _hook(output_dir, device_ids):
            import jax
            jax.devices()
            if device_ids:
                ids = (ctypes.c_int64 * len(device_ids))(*device_ids)
                rc = lib.axon_start_nrt_profile(ids, len(device_ids))
            else:
                rc = lib.axon_start_nrt_profile(None, 0)
            if rc != 0:
                raise RuntimeError(f"axon_start_nrt_profile rc={rc}")
            try:
                yield
            finally:
                n = lib.axon_stop_nrt_profile(str(output_dir).encode())
                print(f"profile: {n} file(s) -> {output_dir}", file=sys.stderr)

        hook = _hook
    except OSError:
        hook = None
    mod = types.ModuleType("antenv.axon_hooks")
    mod.get_axon_ntff_profile_hook = lambda: hook
    mod.set_axon_ntff_profile_hook = lambda h: None
    sys.modules["antenv.axon_hooks"] = mod


_install_ntff_shim()


# ---------------- invocation 1: node tables ----------------
def _build_inv1():
    nc = bacc.Bacc("TRN2", target_bir_lowering=False, debug=False,
                   num_devices=NCORES)
    featT = nc.declare_dram_parameter("featT", [P, NODES_PER_CORE], f32,
                                      isOutput=False)
    W_in = nc.declare_dram_parameter("W", [IN, HD], f32, isOutput=False)
    WT_in = nc.declare_dram_parameter("WT", [HD, IN], f32, isOutput=False)
    Al_in = nc.declare_dram_parameter("Al", [HD, 4], f32, isOutput=False)
    Ar_in = nc.declare_dram_parameter("Ar", [HD, 4], f32, isOutput=False)
    # partition-major outputs: row p holds all its tiles contiguously
    # (fat DMA descriptors); host reshapes back to node-major for free.
    h_out = nc.declare_dram_parameter("h_out", [P, K_WIN * HD], f32,
                                      isOutput=True)
    elr_out = nc.declare_dram_parameter("elr_out", [P, K_WIN * 8], f32,
                                        isOutput=True)

    with tile.TileContext(nc) as tc:
        with tc.tile_pool(name="cst", bufs=1) as cst, \
             tc.tile_pool(name="sb", bufs=3) as sb, \
             tc.tile_pool(name="ps", bufs=3, space="PSUM") as ps, \
             tc.tile_pool(name="psw", bufs=1, space="PSUM") as psw:

            # WLR = [W | Wl | Wr] where Wl = W @ Al, Wr = W @ Ar
            wt_sb = cst.tile([HD, IN], f32, tag="wt")
            nc.sync.dma_start(out=wt_sb[:], in_=WT_in[:])
            al_sb = cst.tile([HD, 4], f32, tag="al")
            nc.sync.dma_start(out=al_sb[:], in_=Al_in[:])
            ar_sb = cst.tile([HD, 4], f32, tag="ar")
            nc.sync.dma_start(out=ar_sb[:], in_=Ar_in[:])

            wlr = cst.tile([IN, 136], f32, tag="wlr")
            nc.sync.dma_start(out=wlr[:, 0:HD], in_=W_in[:])
            wl_ps = psw.tile([IN, 8], f32, tag="wlp")
            nc.tensor.matmul(out=wl_ps[:, 0:4], lhsT=wt_sb[:], rhs=al_sb[:],
                             start=True, stop=True)
            nc.tensor.matmul(out=wl_ps[:, 4:8], lhsT=wt_sb[:], rhs=ar_sb[:],
                             start=True, stop=True)
            nc.scalar.activation(out=wlr[:, 128:136], in_=wl_ps[:],
                                 func=mybir.ActivationFunctionType.Copy)

            CH = 14  # tiles per chunk; 98 = 7 chunks of 14
            n_chunks = NODES_PER_CORE // (P * CH)
            for c in range(n_chunks):
                ft = sb.tile([P, CH * P], f32, tag="ft")
                nc.sync.dma_start(
                    out=ft[:], in_=featT[:, c * CH * P:(c + 1) * CH * P])
                hsb = sb.tile([P, CH * HD], f32, tag="hsb")
                esb = sb.tile([P, CH * 8], f32, tag="esb")
                for t in range(CH):
                    hp = ps.tile([P, 136], f32, tag="hp")
                    # fp32 matmul mode is exact (2 half-speed passes)
                    nc.tensor.matmul(out=hp[:],
                                     lhsT=ft[:, t * P:(t + 1) * P],
                                     rhs=wlr[:], start=True, stop=True)
                    if t % 2 == 0:
                        nc.scalar.activation(
                            out=hsb[:, t * HD:(t + 1) * HD], in_=hp[:, 0:HD],
                            func=mybir.ActivationFunctionType.Copy)
                    else:
                        nc.vector.tensor_copy(hsb[:, t * HD:(t + 1) * HD],
                                              hp[:, 0:HD])
                    nc.vector.tensor_copy(esb[:, t * 8:(t + 1) * 8],
                                          hp[:, 128:136])
                nc.gpsimd.dma_start(
                    out=h_out[:, c * CH * HD:(c + 1) * CH * HD],
                    in_=hsb[:])
                nc.gpsimd.dma_start(
                    out=elr_out[:, c * CH * 8:(c + 1) * CH * 8],
                    in_=esb[:])
    nc.compile()
    return nc


# ---------------- invocation 2: identity-layout edge aggregation ----------
def _build_inv2(Ts, mm_pairs=True, use_gpsimd=True):
    """Ts = per-window tile counts (desc), uniform across cores."""
    Ts = list(Ts)
    CAP = sum(Ts) * P           # hch cols
    CAP4 = sum(Ts) * 4          # el cols
    TMAX = max(max(Ts), 1)
    nc = bacc.Bacc("TRN2", target_bir_lowering=False, debug=False,
                   num_devices=NCORES)
    hsrc = nc.declare_dram_parameter("hsrc", [P, CAP], f32, isOutput=False)
    el_d = nc.declare_dram_parameter("el", [P, CAP4], f32, isOutput=False)
    er_d = nc.declare_dram_parameter("er", [P, K_WIN * 4], f32,
                                     isOutput=False)
    ident_d = nc.declare_dram_parameter("ident", [P, P], f32, isOutput=False)
    bias_in = nc.declare_dram_parameter("bias", [1, HD], f32, isOutput=False)
    out_d = nc.declare_dram_parameter("out", [P, K_WIN * D], f32,
                                      isOutput=True)

    Exp = mybir.ActivationFunctionType.Exp
    Copy = mybir.ActivationFunctionType.Copy
    Add = mybir.AluOpType.add
    Mult = mybir.AluOpType.mult
    Max = mybir.AluOpType.max

    with tile.TileContext(nc) as tc:
        with tc.tile_pool(name="cst", bufs=1) as cst, \
             tc.tile_pool(name="ld", bufs=4) as ld, \
             tc.tile_pool(name="wk", bufs=4) as wk, \
             tc.tile_pool(name="fl", bufs=4) as fl, \
             tc.tile_pool(name="ps", bufs=4, space="PSUM") as ps, \
             tc.tile_pool(name="psb", bufs=1, space="PSUM") as psb:

            # constants
            ident = cst.tile([P, P], f32, tag="ident")
            nc.sync.dma_start(out=ident[:], in_=ident_d[:])
            er_all = cst.tile([P, K_WIN * 4], f32, tag="erall")
            nc.sync.dma_start(out=er_all[:], in_=er_d[:])
            bias_sb = cst.tile([1, HD], f32, tag="brow")
            nc.sync.dma_start(out=bias_sb[:], in_=bias_in[:])
            bias_m = cst.tile([1, D], f32, tag="bm")
            nc.vector.tensor_reduce(
                out=bias_m[:],
                in_=bias_sb[0:1, :].rearrange("p (h d) -> p d h", h=H),
                axis=mybir.AxisListType.X, op=Add)
            nc.vector.tensor_scalar_mul(bias_m[:], bias_m[:], 1.0 / H)
            ones1 = cst.tile([1, P], f32, tag="ones")
            nc.vector.memset(ones1[:], 1.0)
            bias_ps = psb.tile([P, D], f32, tag="bps")
            nc.tensor.matmul(out=bias_ps[:], lhsT=ones1[:], rhs=bias_m[:],
                             start=True, stop=True)
            bias_bc = cst.tile([P, D], f32, tag="bbc")
            nc.vector.tensor_copy(bias_bc[:], bias_ps[:])

            offs = []
            _o = 0
            for _T in Ts:
                offs.append(_o)
                _o += _T

            # software-pipelined loop: window j's DMA + logit/exp chain
            # ("front") is emitted before window k<j's heavy body ("back"),
            # hiding the ACT exp round-trip under the previous multiply.
            pend = {}

            def front(j):
                T = Ts[j]
                off = offs[j]
                KW = T * P
                hch = ld.tile([P, TMAX * P], f32, tag="hch")
                KW2 = (T // 2) * P
                if KW2 > 0:
                    nc.sync.dma_start(
                        out=hch[:, 0:KW2],
                        in_=hsrc[:, off * P:off * P + KW2])
                    nc.sync.dma_start(
                        out=hch[:, KW2:KW],
                        in_=hsrc[:, off * P + KW2:off * P + KW])
                else:
                    nc.sync.dma_start(
                        out=hch[:, 0:KW],
                        in_=hsrc[:, off * P:off * P + KW])
                elch = ld.tile([P, TMAX * 4], f32, tag="elch")
                nc.gpsimd.dma_start(
                    out=elch[:, 0:T * 4],
                    in_=el_d[:, off * 4:off * 4 + T * 4])
                lg = fl.tile([P, TMAX * 4], f32, tag="lg")
                nc.vector.tensor_tensor(
                    out=lg[:, 0:T * 4].rearrange("p (t h) -> p t h", t=T),
                    in0=elch[:, 0:T * 4].rearrange("p (t h) -> p t h", t=T),
                    in1=er_all[:, j * 4:(j + 1) * 4].unsqueeze(1)
                        .to_broadcast([P, T, 4]),
                    op=Add)
                e1 = fl.tile([P, TMAX * 4], f32, tag="e1")
                nc.scalar.activation(out=e1[:, 0:T * 4], in_=lg[:, 0:T * 4],
                                     func=Exp)
                e2 = fl.tile([P, TMAX * 4], f32, tag="e2")
                nc.scalar.activation(out=e2[:, 0:T * 4], in_=lg[:, 0:T * 4],
                                     scale=NEG, func=Exp)
                pend[j] = (hch, e1, e2)

            _first = next((j for j, _T in enumerate(Ts) if _T > 0), None)
            if _first is not None:
                front(_first)
            for k, T in enumerate(Ts):
                if T == 0:
                    # isolated octet: out = bias_mean
                    nc.sync.dma_start(
                        out=out_d[:, k * D:(k + 1) * D], in_=bias_bc[:])
                    continue
                jn = next((j for j in range(k + 1, len(Ts)) if Ts[j] > 0),
                          None)
                if jn is not None:
                    front(jn)
                hch, e1, e2 = pend.pop(k)
                eng_a = nc.gpsimd if use_gpsimd else nc.vector

                expE = fl.tile([P, TMAX * 4], f32, tag="expE")
                eng_a.tensor_tensor(out=expE[:, 0:T * 4],
                                    in0=e1[:, 0:T * 4],
                                    in1=e2[:, 0:T * 4], op=Max)

                # s, r per head
                s4 = fl.tile([P, 4], f32, tag="s4")
                nc.vector.tensor_reduce(
                    out=s4[:],
                    in_=expE[:, 0:T * 4].rearrange("p (t h) -> p h t", t=T),
                    axis=mybir.AxisListType.X, op=Add)
                # host plants el=ln(1e-6) in each row's first padding slot,
                # so s >= 1e-6 always and no clamp op is needed here
                r4 = fl.tile([P, 4], f32, tag="r4")
                nc.vector.reciprocal(r4[:], s4[:])

                # wmsg = hch * expE (broadcast over D), split in two halves
                # so PE matmuls can start after the first half completes
                wmsg = wk.tile([P, TMAX * P], f32, tag="wmsg")
                Ta = max(1, T // 2)
                for (ta, tb) in ((0, Ta), (Ta, T)):
                    tn = tb - ta
                    if tn <= 0:
                        continue
                    nc.vector.tensor_tensor(
                        out=wmsg[:, ta * P:tb * P].rearrange(
                            "p (t h d) -> p t h d", t=tn, h=H),
                        in0=hch[:, ta * P:tb * P].rearrange(
                            "p (t h d) -> p t h d", t=tn, h=H),
                        in1=expE[:, ta * 4:tb * 4].rearrange(
                            "p (t h) -> p t h", t=tn).unsqueeze(3)
                            .to_broadcast([P, tn, H, D]),
                        op=Mult)

                # U = sum_t wmsg_t  (identity matmul, PSUM accumulate;
                # 0.25 head-mean folded into ident)
                if mm_pairs:
                    best, ndve = None, 0
                    for n in range(0, T):
                        dve_t = 2220 + 1350 + (230 if n else 0) + 230 * n
                        pe_t = 244 * (T - n)
                        m = max(dve_t, pe_t)
                        if best is None or m < best:
                            best, ndve = m, n
                else:
                    ndve = 0
                npe = T - ndve
                acc = ps.tile([P, HD], f32, tag="acc")
                for t in range(npe):
                    nc.tensor.matmul(
                        out=acc[:], lhsT=ident[:],
                        rhs=wmsg[:, t * P:(t + 1) * P],
                        start=(t == 0), stop=(t == npe - 1))
                if ndve > 0:
                    if ndve == 1:
                        dsrc = wmsg[:, npe * P:(npe + 1) * P]
                    else:
                        udve = fl.tile([P, HD], f32, tag="udve")
                        nc.vector.tensor_tensor(
                            out=udve[:], in0=wmsg[:, npe * P:(npe + 1) * P],
                            in1=wmsg[:, (npe + 1) * P:(npe + 2) * P], op=Add)
                        for t in range(npe + 2, T):
                            nc.vector.tensor_tensor(
                                out=udve[:], in0=udve[:],
                                in1=wmsg[:, t * P:(t + 1) * P], op=Add)
                        dsrc = udve[:]
                    u128 = fl.tile([P, HD], f32, tag="u128")
                    # u128 = (dsrc * 0.25) + acc (0.25 fold lives in ident)
                    nc.vector.scalar_tensor_tensor(
                        out=u128[:], in0=dsrc, scalar=0.25, in1=acc[:],
                        op0=Mult, op1=Add)
                    usrc = u128
                else:
                    usrc = acc

                # flush: out = sum_h (U_h * r_h) + bias_mean
                un = fl.tile([P, HD], f32, tag="un")
                for hh in range(H):
                    nc.scalar.activation(
                        out=un[:, hh * D:(hh + 1) * D],
                        in_=usrc[:, hh * D:(hh + 1) * D],
                        func=Copy, scale=r4[:, hh:hh + 1])
                red = fl.tile([P, D], f32, tag="red")
                nc.vector.tensor_reduce(
                    out=red[:],
                    in_=un[:].rearrange("p (h d) -> p d h", h=H),
                    axis=mybir.AxisListType.X, op=Add)
                outt = fl.tile([P, D], f32, tag="outt")
                eng_a.tensor_tensor(out=outt[:], in0=red[:],
                                    in1=bias_bc[:], op=Add)
                nc.gpsimd.dma_start(out=out_d[:, k * D:(k + 1) * D],
                                    in_=outt[:])
    nc.compile()
    return nc


_INV1 = None
_INV2 = {}
LAST_EXEC_NS = None
LAST_EXEC_NS1 = None
LAST_EXEC_NS2 = None
_TRACE = bool(os.environ.get("GAT_TRACE"))


def kernel(feat, W, attn_l, attn_r, bias, src, dst):
    global _INV1, LAST_EXEC_NS, LAST_EXEC_NS1, LAST_EXEC_NS2
    feat = np.asarray(feat, dtype=np.float32)
    W = np.asarray(W, dtype=np.float32)
    attn_l = np.asarray(attn_l, dtype=np.float32)
    attn_r = np.asarray(attn_r, dtype=np.float32)
    bias = np.asarray(bias, dtype=np.float32)
    src = np.asarray(src, dtype=np.int32)
    dst = np.asarray(dst, dtype=np.int32)

    # ---------------- host: layout-only prep ----------------
    featT = np.zeros((IN, N_PAD), dtype=np.float32)
    featT[:, :N] = np.ascontiguousarray(feat.T)
    WT = np.ascontiguousarray(W.T)
    Al = np.zeros((HD, H), dtype=np.float32)
    Ar = np.zeros((HD, H), dtype=np.float32)
    for h in range(H):
        Al[h * D:(h + 1) * D, h] = attn_l[h]
        Ar[h * D:(h + 1) * D, h] = attn_r[h]

    # ---------------- inv-1: node tables ----------------
    if _INV1 is None:
        _INV1 = _build_inv1()
    in1 = []
    for c in range(NCORES):
        sl = slice(c * NODES_PER_CORE, (c + 1) * NODES_PER_CORE)
        in1.append({"featT": np.ascontiguousarray(featT[:, sl]),
                    "W": W, "WT": WT, "Al": Al, "Ar": Ar})
    res1 = run_bass_kernel_spmd(_INV1, in1, core_ids=list(range(NCORES)),
                                trace=_TRACE)
    LAST_EXEC_NS1 = res1.exec_time_ns
    h_full = np.concatenate(
        [r["h_out"].reshape(P, K_WIN, HD).transpose(1, 0, 2)
         .reshape(NODES_PER_CORE, HD) for r in res1.results], axis=0)
    elr_full = np.concatenate(
        [r["elr_out"].reshape(P, K_WIN, 8).transpose(1, 0, 2)
         .reshape(NODES_PER_CORE, 8) for r in res1.results], axis=0)

    # ---------------- host: identity-layout slotting (index ops only) -----
    deg = np.bincount(dst, minlength=N_PAD).astype(np.int64)
    order = np.argsort(-deg, kind="stable")
    rank = np.empty(N_PAD, dtype=np.int64)
    rank[order] = np.arange(N_PAD)
    k_of = rank >> 10                  # octet index (node -> window)
    within = rank & 1023
    c_of = within >> 7
    c_of = np.where(k_of & 1 == 1, NCORES - 1 - c_of, c_of)  # snake
    p_of = within & 127

    Ts = deg[order[::1024]]            # max deg per octet (sorted desc)
    Ts = np.maximum(Ts, 0)
    # schedule: start with the 8 smallest windows (fast pipeline ramp),
    # then big -> small
    sched = np.concatenate([np.arange(K_WIN - 8, K_WIN),
                            np.arange(K_WIN - 8)])
    oct_pos = np.empty(K_WIN, dtype=np.int64)
    oct_pos[sched] = np.arange(K_WIN)
    Ts_s = Ts[sched]
    key = tuple(int(t) for t in Ts_s)
    tile_off = np.zeros(K_WIN + 1, dtype=np.int64)
    np.cumsum(Ts_s, out=tile_off[1:])
    CAP = int(tile_off[-1]) * P
    w_of = oct_pos[k_of]               # node -> window position in schedule

    # per-edge slot: t = running count within dst node
    perm = np.argsort(dst, kind="stable")
    dstp = dst[perm]
    srcp = src[perm]
    estart = np.zeros(N_PAD + 1, dtype=np.int64)
    np.cumsum(np.bincount(dstp, minlength=N_PAD), out=estart[1:])
    t_of = np.arange(E, dtype=np.int64) - estart[dstp]
    ce = c_of[dstp]
    pe = p_of[dstp]
    tile_e = tile_off[w_of[dstp]] + t_of

    # gathers (pure data movement)
    n_tiles = int(tile_off[-1])
    hsrc_lay = np.zeros((NCORES, P, n_tiles, HD), dtype=np.float32)
    hsrc_lay[ce, pe, tile_e, :] = h_full[srcp]
    el_lay = np.full((NCORES, P, n_tiles, 4), PAD_LOGIT, dtype=np.float32)
    el_lay[ce, pe, tile_e, :] = elr_full[srcp][:, 0:4]
    # plant el=ln(1e-6) in each row's first padding slot so s >= 1e-6
    # (replaces the device-side max(s, eps) clamp; harmless: adds 1e-6
    # to s where real s >= ~0.2, and gives empty rows s=1e-6 with U=0)
    # value -69.08 = 5*ln(1e-6): through max(exp(x), exp(0.2x)) this
    # contributes exactly ~1e-6
    has_pad = deg < Ts_s[w_of]
    el_lay[c_of[has_pad], p_of[has_pad],
           (tile_off[w_of] + deg)[has_pad], :] = -69.07755

    er_lay = np.zeros((NCORES, P, K_WIN, 4), dtype=np.float32)
    nodes = np.arange(N_PAD)
    er_lay[c_of, p_of, w_of, :] = elr_full[nodes][:, 4:8]
    ident = (0.25 * np.eye(P)).astype(np.float32)

    # ---------------- inv-2: edge aggregation ----------------
    if key not in _INV2:
        _INV2[key] = _build_inv2(key, mm_pairs=True, use_gpsimd=False)
    in2 = []
    for c in range(NCORES):
        in2.append({"hsrc": hsrc_lay[c].reshape(P, CAP),
                    "el": el_lay[c].reshape(P, n_tiles * 4),
                    "er": er_lay[c].reshape(P, K_WIN * 4),
                    "ident": ident,
                    "bias": bias.reshape(1, HD)})
    res2 = run_bass_kernel_spmd(_INV2[key], in2, core_ids=list(range(NCORES)),
                                trace=_TRACE)
    LAST_EXEC_NS2 = res2.exec_time_ns
    if LAST_EXEC_NS1 is not None and LAST_EXEC_NS2 is not None:
        LAST_EXEC_NS = LAST_EXEC_NS1 + LAST_EXEC_NS2
    out_full = np.zeros((N_PAD, D), dtype=np.float32)
    res_arr = np.stack([r["out"].reshape(P, K_WIN, D)
                        for r in res2.results])  # [c, p, k, d]
    out_full[nodes] = res_arr[c_of, p_of, w_of, :]
    return np.ascontiguousarray(out_full[:N])

